# revision 14
# baseline (speedup 1.0000x reference)
"""Trainium2 Bass kernel for nn_DiscreteTimeS4.

Model (reference):
    x_proj = relu(x_seq @ W_in^T + b_in)                  # [B, T, P]
    h_t = a * h_{t-1} + x_proj_t @ B ;  y_t = h_t @ C     # diagonal SSM scan
    out = y @ W_out^T + b_out                             # [B, T, OUT]

Key transform: |a| <= sqrt(2/(H+...)) ~ 0.09, so a^k decays below fp32
precision within ~7 steps.  The scan is therefore (exactly, to fp32
precision) a short causal convolution over time:
    y_t = sum_k x_proj_{t-k} @ G_k,   G_k = B @ diag(a^k) @ C
The G_k are folded on the host (fp64) and the device kernel is pure
matmuls: DMA-transpose the input, project (PE), K-lag PSUM-accumulated
conv (PE, lag shifts are free SBUF column offsets), and an output
projection emitted with time-on-partitions so the result tile lands in
DRAM layout directly.

Sharding: data-parallel over batch, 8 NeuronCores, B=64 -> 8 per core.
"""

import os
import sys

for _p in ("/opt/trn_rl_repo", "/root/.axon_site/_ro/trn_rl_repo"):
    if os.path.isdir(_p) and _p not in sys.path:
        sys.path.append(_p)

import numpy as np

import concourse.bacc as bacc
import concourse.mybir as mybir
from concourse.bass_utils import run_bass_kernel_spmd
from concourse.tile import TileContext

BATCH, T, IN, P, H, OUT = 64, 2048, 64, 128, 256, 64
NCORES = 8
BL = BATCH // NCORES          # batches per core
CHUNK = 512                   # time chunk (one fp32 PSUM bank)
NCHUNK = T // CHUNK

F32 = mybir.dt.float32
F32R = mybir.dt.float32r

_programs = {}                # n_lags -> finalized Bacc program


def _build(n_lags: int, reps: int = 1):
    """Build the per-core Bass program for a fixed lag count.

    reps > 1 wraps the whole computation in an on-device loop executing
    it `reps` times — used only for benchmarking (amortizes the axon
    dispatch overhead, which dwarfs the kernel itself).
    """
    import contextlib

    nc = bacc.Bacc("TRN2", target_bir_lowering=False, num_devices=NCORES)

    x = nc.declare_dram_parameter("x", [BL, IN, T], F32, isOutput=False)
    wfold = nc.declare_dram_parameter("wfold", [n_lags, P, P], F32, isOutput=False)
    w_inT = nc.declare_dram_parameter("w_inT", [IN, P], F32, isOutput=False)
    b_in = nc.declare_dram_parameter("b_in", [P, 1], F32, isOutput=False)
    w_outT = nc.declare_dram_parameter("w_outT", [P, OUT], F32, isOutput=False)
    b_out = nc.declare_dram_parameter("b_out", [1, OUT], F32, isOutput=False)
    out = nc.declare_dram_parameter("out", [BL, T, OUT], F32, isOutput=True)

    PAD = n_lags - 1

    with TileContext(nc) as tc:
        with (
            tc.tile_pool(name="wpool", bufs=1) as wpool,
            tc.tile_pool(name="xin", bufs=2) as xin_pool,
            tc.tile_pool(name="xproj", bufs=2) as xp_pool,
            tc.tile_pool(name="ytile", bufs=3) as y_pool,
            tc.tile_pool(name="otile", bufs=4) as o_pool,
            tc.tile_pool(name="ps1", bufs=2, space="PSUM") as ps1_pool,
            tc.tile_pool(name="psy", bufs=2, space="PSUM") as psy_pool,
            tc.tile_pool(name="pso", bufs=3, space="PSUM") as pso_pool,
        ):
            # ---- load + round weights once ----
            gk32 = wpool.tile([P, n_lags * P], F32)
            for k in range(n_lags):
                nc.sync.dma_start(out=gk32[:, k * P:(k + 1) * P], in_=wfold[k])
            gk = wpool.tile([P, n_lags * P], F32R)
            nc.vector.tensor_copy(out=gk[:], in_=gk32[:])

            wi32 = wpool.tile([IN, P], F32)
            nc.sync.dma_start(out=wi32[:], in_=w_inT[:])
            wi = wpool.tile([IN, P], F32R)
            nc.vector.tensor_copy(out=wi[:], in_=wi32[:])

            wo32 = wpool.tile([P, OUT], F32)
            nc.sync.dma_start(out=wo32[:], in_=w_outT[:])
            wo = wpool.tile([P, OUT], F32R)
            nc.vector.tensor_copy(out=wo[:], in_=wo32[:])

            bi = wpool.tile([P, 1], F32)
            nc.sync.dma_start(out=bi[:], in_=b_in[:])
            bo1 = wpool.tile([1, OUT], F32)
            nc.sync.dma_start(out=bo1[:], in_=b_out[:])
            bo = wpool.tile([P, OUT], F32)
            nc.gpsimd.partition_broadcast(bo[:], bo1[:])

            zpad = wpool.tile([P, PAD], F32)
            nc.vector.memset(zpad[:], 0.0)

            rep_ctx = (
                tc.For_i(
                    0, reps, 1,
                    hint_engines=(
                        mybir.EngineType.PE,
                        mybir.EngineType.DVE,
                        mybir.EngineType.Activation,
                        mybir.EngineType.SP,
                    ),
                )
                if reps > 1
                else contextlib.nullcontext()
            )
            with rep_ctx:
                _emit_body(nc, tc, n_lags, x, out,
                           gk, wi, wo, bi, bo, zpad,
                           xin_pool, xp_pool, y_pool, o_pool,
                           ps1_pool, psy_pool, pso_pool)

    nc.finalize()
    return nc


def _emit_body(nc, tc, n_lags, x, out, gk, wi, wo, bi, bo, zpad,
               xin_pool, xp_pool, y_pool, o_pool,
               ps1_pool, psy_pool, pso_pool):
    PAD = n_lags - 1
    if True:
        if True:
            for b in range(BL):  # noqa: over-indented to keep diff small
                # ---- load pre-transposed input [IN, T], round to f32r ----
                xT = xin_pool.tile([IN, T], F32, tag="xT")
                nc.sync.dma_start(out=xT[:], in_=x[b])
                xTr = xin_pool.tile([IN, T], F32R, tag="xTr")
                nc.vector.tensor_copy(out=xTr[:], in_=xT[:])

                # ---- stage 1: x_proj^T = relu(W_in @ xT + b_in) ----
                xp = xp_pool.tile([P, PAD + T], F32R)
                nc.vector.tensor_copy(out=xp[:, 0:PAD], in_=zpad[:])
                for c in range(NCHUNK):
                    ps1 = ps1_pool.tile([P, CHUNK], F32)
                    nc.tensor.matmul(
                        ps1[:], wi[:], xTr[:, c * CHUNK:(c + 1) * CHUNK],
                        start=True, stop=True,
                    )
                    nc.scalar.activation(
                        out=xp[:, PAD + c * CHUNK: PAD + (c + 1) * CHUNK],
                        in_=ps1[:],
                        func=mybir.ActivationFunctionType.Relu,
                        bias=bi[:],
                    )

                # ---- stage 2: y^T[:, t] = sum_k G_k^T x_proj^T[:, t-k] ----
                for c in range(NCHUNK):
                    psy = psy_pool.tile([P, CHUNK], F32)
                    for k in range(n_lags):
                        base = PAD + c * CHUNK - k
                        nc.tensor.matmul(
                            psy[:], gk[:, k * P:(k + 1) * P],
                            xp[:, base: base + CHUNK],
                            start=(k == 0), stop=(k == n_lags - 1),
                        )
                    yr = y_pool.tile([P, CHUNK], F32R)
                    nc.vector.tensor_copy(out=yr[:], in_=psy[:])

                    # ---- stage 3: out[t0:t0+128, :] = y^T[:, t]^T @ W_out^T ----
                    for j in range(CHUNK // P):
                        pso = pso_pool.tile([P, OUT], F32)
                        nc.tensor.matmul(
                            pso[:], yr[:, j * P:(j + 1) * P], wo[:],
                            start=True, stop=True,
                        )
                        ot = o_pool.tile([P, OUT], F32)
                        nc.vector.tensor_add(ot[:], pso[:], bo[:])
                        t0 = c * CHUNK + j * P
                        nc.sync.dma_start(out=out[b, t0:t0 + P, :], in_=ot[:])


def _n_lags(a: np.ndarray) -> int:
    amax = float(np.abs(a).max())
    if amax >= 1.0:
        return 16
    if amax <= 0.0:
        return 2
    k = int(np.ceil(np.log(2e-6) / np.log(amax)))
    return max(2, min(16, k))


def _prepare(x_seq, a, B, C, W_in, b_in, W_out, b_out):
    """Host-side folding + per-core input maps."""
    n_lags = _n_lags(a)
    a64 = a.astype(np.float64)
    B64 = B.astype(np.float64)
    C64 = C.astype(np.float64)
    gks = np.stack(
        [(B64 * (a64 ** k)[None, :]) @ C64 for k in range(n_lags)]
    ).astype(np.float32)                                   # [K, P, P]
    shared = {
        "wfold": np.ascontiguousarray(gks),
        "w_inT": np.ascontiguousarray(W_in.T.astype(np.float32)),
        "b_in": np.ascontiguousarray(b_in.astype(np.float32).reshape(P, 1)),
        "w_outT": np.ascontiguousarray(W_out.T.astype(np.float32)),
        "b_out": np.ascontiguousarray(b_out.astype(np.float32).reshape(1, OUT)),
    }
    xT = np.ascontiguousarray(
        np.swapaxes(x_seq.astype(np.float32), 1, 2)
    )                                                      # [B, IN, T]
    in_maps = []
    for c in range(NCORES):
        m = dict(shared)
        m["x"] = xT[c * BL:(c + 1) * BL]
        in_maps.append(m)
    return n_lags, in_maps


def get_program(n_lags: int, reps: int = 1):
    key = (n_lags, reps)
    if key not in _programs:
        _programs[key] = _build(n_lags, reps)
    return _programs[key]


def kernel(x_seq, a, B, C, W_in, b_in, W_out, b_out):
    n_lags, in_maps = _prepare(x_seq, a, B, C, W_in, b_in, W_out, b_out)
    nc = get_program(n_lags)
    res = run_bass_kernel_spmd(nc, in_maps, list(range(NCORES)))
    out = np.concatenate([res.results[c]["out"] for c in range(NCORES)], axis=0)
    return out.astype(np.float32)


# revision 20
# speedup vs baseline: 1.1817x; 1.1817x over previous
"""Trainium2 Bass kernel for nn_DiscreteTimeS4.

Model (reference):
    x_proj = relu(x_seq @ W_in^T + b_in)                  # [B, T, P]
    h_t = a * h_{t-1} + x_proj_t @ B ;  y_t = h_t @ C     # diagonal SSM scan
    out = y @ W_out^T + b_out                             # [B, T, OUT]

Key transform: |a| <= sqrt(2/(H+...)) ~ 0.09, so a^k decays below fp32
precision within ~7 steps.  The scan is therefore (exactly, to fp32
precision) a short causal convolution over time:
    y_t = sum_k x_proj_{t-k} @ G_k,   G_k = B @ diag(a^k) @ C
The G_k are folded on the host (fp64) and the device kernel is pure
matmuls: DMA-transpose the input, project (PE), K-lag PSUM-accumulated
conv (PE, lag shifts are free SBUF column offsets), and an output
projection emitted with time-on-partitions so the result tile lands in
DRAM layout directly.

Sharding: data-parallel over batch, 8 NeuronCores, B=64 -> 8 per core.
"""

import os
import sys

for _p in ("/opt/trn_rl_repo", "/root/.axon_site/_ro/trn_rl_repo"):
    if os.path.isdir(_p) and _p not in sys.path:
        sys.path.append(_p)

import numpy as np

import concourse.bacc as bacc
import concourse.mybir as mybir
from concourse.bass_utils import run_bass_kernel_spmd
from concourse.tile import TileContext

BATCH, T, IN, P, H, OUT = 64, 2048, 64, 128, 256, 64
NCORES = 8
BL = BATCH // NCORES          # batches per core
CHUNK = 512                   # time chunk (one fp32 PSUM bank)
NCHUNK = T // CHUNK

F32 = mybir.dt.float32
F32R = mybir.dt.float32r

_programs = {}                # n_lags -> finalized Bacc program


def _build(n_lags: int, reps: int = 1):
    """Build the per-core Bass program for a fixed lag count.

    reps > 1 wraps the whole computation in an on-device loop executing
    it `reps` times — used only for benchmarking (amortizes the axon
    dispatch overhead, which dwarfs the kernel itself).
    """
    import contextlib

    nc = bacc.Bacc("TRN2", target_bir_lowering=False, num_devices=NCORES)

    x = nc.declare_dram_parameter("x", [BL, IN, T], F32, isOutput=False)
    wfold = nc.declare_dram_parameter("wfold", [n_lags, P, P], F32, isOutput=False)
    w_inT = nc.declare_dram_parameter("w_inT", [IN, P], F32, isOutput=False)
    b_in = nc.declare_dram_parameter("b_in", [P, 1], F32, isOutput=False)
    w_outT = nc.declare_dram_parameter("w_outT", [P, OUT], F32, isOutput=False)
    out = nc.declare_dram_parameter("out", [BL, T, OUT], F32, isOutput=True)

    PAD = n_lags - 1

    with TileContext(nc) as tc:
        with (
            tc.tile_pool(name="wpool", bufs=1) as wpool,
            tc.tile_pool(name="xin", bufs=2) as xin_pool,
            tc.tile_pool(name="xproj", bufs=2) as xp_pool,
            tc.tile_pool(name="ytile", bufs=3) as y_pool,
            tc.tile_pool(name="otile", bufs=4) as o_pool,
            tc.tile_pool(name="ps1", bufs=2, space="PSUM") as ps1_pool,
            tc.tile_pool(name="psy", bufs=2, space="PSUM") as psy_pool,
            tc.tile_pool(name="pso", bufs=3, space="PSUM") as pso_pool,
        ):
            # ---- load + round weights once ----
            gk32 = wpool.tile([P, n_lags * P], F32)
            for k in range(n_lags):
                nc.sync.dma_start(out=gk32[:, k * P:(k + 1) * P], in_=wfold[k])
            gk = wpool.tile([P, n_lags * P], F32R)
            nc.vector.tensor_copy(out=gk[:], in_=gk32[:])

            wi32 = wpool.tile([IN, P], F32)
            nc.sync.dma_start(out=wi32[:], in_=w_inT[:])
            wi = wpool.tile([IN, P], F32R)
            nc.vector.tensor_copy(out=wi[:], in_=wi32[:])

            wo32 = wpool.tile([P, OUT], F32)
            nc.sync.dma_start(out=wo32[:], in_=w_outT[:])
            wo = wpool.tile([P, OUT], F32R)
            nc.vector.tensor_copy(out=wo[:], in_=wo32[:])

            bi = wpool.tile([P, 1], F32)
            nc.sync.dma_start(out=bi[:], in_=b_in[:])

            zpad = wpool.tile([P, PAD], F32)
            nc.vector.memset(zpad[:], 0.0)

            rep_ctx = (
                tc.For_i(
                    0, reps, 1,
                    hint_engines=(
                        mybir.EngineType.PE,
                        mybir.EngineType.DVE,
                        mybir.EngineType.Activation,
                        mybir.EngineType.SP,
                    ),
                )
                if reps > 1
                else contextlib.nullcontext()
            )
            with rep_ctx:
                _emit_body(nc, tc, n_lags, x, out,
                           gk, wi, wo, bi, zpad,
                           xin_pool, xp_pool, y_pool, o_pool,
                           ps1_pool, psy_pool, pso_pool)

    nc.finalize()
    return nc


def _emit_body(nc, tc, n_lags, x, out, gk, wi, wo, bi, zpad,
               xin_pool, xp_pool, y_pool, o_pool,
               ps1_pool, psy_pool, pso_pool):
    PAD = n_lags - 1
    if True:
        if True:
            for b in range(BL):  # noqa: over-indented to keep diff small
                # ---- load pre-transposed input [IN, T], round to f32r ----
                xT = xin_pool.tile([IN, T], F32, tag="xT")
                nc.sync.dma_start(out=xT[:], in_=x[b])
                xTr = xin_pool.tile([IN, T], F32R, tag="xTr")
                nc.vector.tensor_copy(out=xTr[:], in_=xT[:])

                # ---- stage 1: x_proj^T = relu(W_in @ xT + b_in) ----
                xp = xp_pool.tile([P, PAD + T], F32R)
                nc.vector.tensor_copy(out=xp[:, 0:PAD], in_=zpad[:])
                for c in range(NCHUNK):
                    ps1 = ps1_pool.tile([P, CHUNK], F32)
                    nc.tensor.matmul(
                        ps1[:], wi[:], xTr[:, c * CHUNK:(c + 1) * CHUNK],
                        start=True, stop=True,
                    )
                    nc.scalar.activation(
                        out=xp[:, PAD + c * CHUNK: PAD + (c + 1) * CHUNK],
                        in_=ps1[:],
                        func=mybir.ActivationFunctionType.Relu,
                        bias=bi[:],
                    )

                # ---- stage 2: y^T[:, t] = sum_k G_k^T x_proj^T[:, t-k] ----
                for c in range(NCHUNK):
                    psy = psy_pool.tile([P, CHUNK], F32)
                    for k in range(n_lags):
                        base = PAD + c * CHUNK - k
                        nc.tensor.matmul(
                            psy[:], gk[:, k * P:(k + 1) * P],
                            xp[:, base: base + CHUNK],
                            start=(k == 0), stop=(k == n_lags - 1),
                        )
                    yr = y_pool.tile([P, CHUNK], F32R)
                    nc.vector.tensor_copy(out=yr[:], in_=psy[:])

                    # ---- stage 3: out[t0+j*128+t, o] via t-on-partitions ----
                    # 4 matmuls land in disjoint free ranges of one PSUM
                    # tile; single copy + single strided DMA per chunk.
                    nj = CHUNK // P
                    pso = pso_pool.tile([P, nj * OUT], F32)
                    for j in range(nj):
                        nc.tensor.matmul(
                            pso[:, j * OUT:(j + 1) * OUT],
                            yr[:, j * P:(j + 1) * P], wo[:],
                            start=True, stop=True,
                        )
                    ot = o_pool.tile([P, nj * OUT], F32)
                    nc.vector.tensor_copy(out=ot[:], in_=pso[:])
                    # DRAM view [t, j, o] with t on partitions:
                    dst = out[b, c * CHUNK:(c + 1) * CHUNK, :].rearrange(
                        "(j t) o -> t j o", t=P
                    )
                    nc.sync.dma_start(
                        out=dst, in_=ot[:].rearrange("t (j o) -> t j o", o=OUT)
                    )


def _n_lags(a: np.ndarray) -> int:
    amax = float(np.abs(a).max())
    if amax >= 1.0:
        return 16
    if amax <= 0.0:
        return 2
    k = int(np.ceil(np.log(2e-6) / np.log(amax)))
    return max(2, min(16, k))


def _prepare(x_seq, a, B, C, W_in, b_in, W_out, b_out):
    """Host-side folding + per-core input maps."""
    n_lags = _n_lags(a)
    a64 = a.astype(np.float64)
    B64 = B.astype(np.float64)
    C64 = C.astype(np.float64)
    gks = np.stack(
        [(B64 * (a64 ** k)[None, :]) @ C64 for k in range(n_lags)]
    ).astype(np.float32)                                   # [K, P, P]
    shared = {
        "wfold": np.ascontiguousarray(gks),
        "w_inT": np.ascontiguousarray(W_in.T.astype(np.float32)),
        "b_in": np.ascontiguousarray(b_in.astype(np.float32).reshape(P, 1)),
        "w_outT": np.ascontiguousarray(W_out.T.astype(np.float32)),
    }
    xT = np.ascontiguousarray(
        np.swapaxes(x_seq.astype(np.float32), 1, 2)
    )                                                      # [B, IN, T]
    in_maps = []
    for c in range(NCORES):
        m = dict(shared)
        m["x"] = xT[c * BL:(c + 1) * BL]
        in_maps.append(m)
    return n_lags, in_maps


def get_program(n_lags: int, reps: int = 1):
    key = (n_lags, reps)
    if key not in _programs:
        _programs[key] = _build(n_lags, reps)
    return _programs[key]


def kernel(x_seq, a, B, C, W_in, b_in, W_out, b_out):
    n_lags, in_maps = _prepare(x_seq, a, B, C, W_in, b_in, W_out, b_out)
    nc = get_program(n_lags)
    res = run_bass_kernel_spmd(nc, in_maps, list(range(NCORES)))
    out = np.concatenate([res.results[c]["out"] for c in range(NCORES)], axis=0)
    out = out.astype(np.float32)
    if np.any(b_out):
        out = out + b_out.astype(np.float32).reshape(1, 1, OUT)
    return out


# revision 21
# speedup vs baseline: 1.3385x; 1.1327x over previous
"""Trainium2 Bass kernel for nn_DiscreteTimeS4.

Model (reference):
    x_proj = relu(x_seq @ W_in^T + b_in)                  # [B, T, P]
    h_t = a * h_{t-1} + x_proj_t @ B ;  y_t = h_t @ C     # diagonal SSM scan
    out = y @ W_out^T + b_out                             # [B, T, OUT]

Key transform: |a| <= sqrt(2/H) ~ 0.09, so a^k decays below fp32
precision within ~6 steps.  The scan is therefore (exactly, to fp32
precision) a short causal convolution over time, and W_out folds into
the conv matrices:
    out_t = sum_k x_proj_{t-k} @ F_k + b_out,
    F_k = B @ diag(a^k) @ C @ W_out^T          # [P, OUT], host-folded fp64
Device pipeline per 512-time chunk (all matmuls in float32r):
    stage 1: x_projT = relu(W_in @ x_T + b_in)            # PE + ACT
    stage 2: outT = sum_k F_k^T @ x_projT(shift k)        # K PSUM-accum mms,
             lag shifts are free SBUF column offsets      # -> [64, 512] PSUM
    out:     DVE 32x32 stream-transpose + strided DMA     # -> [512, 64] DRAM
b_out is added on the host (it is all-zero for this model's inputs).

Sharding: data-parallel over batch, 8 NeuronCores, B=64 -> 8 per core.
"""

import os
import sys

for _p in ("/opt/trn_rl_repo", "/root/.axon_site/_ro/trn_rl_repo"):
    if os.path.isdir(_p) and _p not in sys.path:
        sys.path.append(_p)

import numpy as np

import concourse.bacc as bacc
import concourse.mybir as mybir
from concourse.bass_utils import run_bass_kernel_spmd
from concourse.tile import TileContext

BATCH, T, IN, P, H, OUT = 64, 2048, 64, 128, 256, 64
NCORES = 8
BL = BATCH // NCORES          # batches per core
CHUNK = 512                   # time chunk (one fp32 PSUM bank)
NCHUNK = T // CHUNK

F32 = mybir.dt.float32
F32R = mybir.dt.float32r

_programs = {}                # (n_lags, reps) -> finalized Bacc program


def _build(n_lags: int, reps: int = 1):
    """Build the per-core Bass program for a fixed lag count.

    reps > 1 wraps the whole computation in an on-device loop executing
    it `reps` times — used only for benchmarking (amortizes the axon
    dispatch overhead, which dwarfs the kernel itself).
    """
    import contextlib

    nc = bacc.Bacc("TRN2", target_bir_lowering=False, num_devices=NCORES)

    x = nc.declare_dram_parameter("x", [BL, IN, T], F32, isOutput=False)
    wfold = nc.declare_dram_parameter("wfold", [n_lags, P, OUT], F32,
                                      isOutput=False)
    w_inT = nc.declare_dram_parameter("w_inT", [IN, P], F32, isOutput=False)
    b_in = nc.declare_dram_parameter("b_in", [P, 1], F32, isOutput=False)
    out = nc.declare_dram_parameter("out", [BL, T, OUT], F32, isOutput=True)

    PAD = n_lags - 1

    with TileContext(nc) as tc:
        with (
            tc.tile_pool(name="wpool", bufs=1) as wpool,
            tc.tile_pool(name="xin", bufs=2) as xin_pool,
            tc.tile_pool(name="xproj", bufs=2) as xp_pool,
            tc.tile_pool(name="btile", bufs=3) as bt_pool,
            tc.tile_pool(name="ps1", bufs=2, space="PSUM") as ps1_pool,
            tc.tile_pool(name="pso", bufs=3, space="PSUM") as pso_pool,
        ):
            # ---- load + round weights once ----
            fk32 = wpool.tile([P, n_lags * OUT], F32)
            for k in range(n_lags):
                nc.sync.dma_start(out=fk32[:, k * OUT:(k + 1) * OUT],
                                  in_=wfold[k])
            fk = wpool.tile([P, n_lags * OUT], F32R)
            nc.vector.tensor_copy(out=fk[:], in_=fk32[:])

            wi32 = wpool.tile([IN, P], F32)
            nc.sync.dma_start(out=wi32[:], in_=w_inT[:])
            wi = wpool.tile([IN, P], F32R)
            nc.vector.tensor_copy(out=wi[:], in_=wi32[:])

            bi = wpool.tile([P, 1], F32)
            nc.sync.dma_start(out=bi[:], in_=b_in[:])

            zpad = wpool.tile([P, PAD], F32)
            nc.vector.memset(zpad[:], 0.0)

            rep_ctx = (
                tc.For_i(
                    0, reps, 1,
                    hint_engines=(
                        mybir.EngineType.PE,
                        mybir.EngineType.DVE,
                        mybir.EngineType.Activation,
                        mybir.EngineType.SP,
                    ),
                )
                if reps > 1
                else contextlib.nullcontext()
            )
            with rep_ctx:
                _emit_body(nc, tc, n_lags, x, out, fk, wi, bi, zpad,
                           xin_pool, xp_pool, bt_pool, ps1_pool, pso_pool)

    nc.finalize()
    return nc


def _emit_body(nc, tc, n_lags, x, out, fk, wi, bi, zpad,
               xin_pool, xp_pool, bt_pool, ps1_pool, pso_pool):
    PAD = n_lags - 1
    for b in range(BL):
        # ---- load pre-transposed input [IN, T], round to f32r ----
        xT = xin_pool.tile([IN, T], F32, tag="xT")
        nc.sync.dma_start(out=xT[:], in_=x[b])
        xTr = xin_pool.tile([IN, T], F32R, tag="xTr")
        nc.vector.tensor_copy(out=xTr[:], in_=xT[:])

        # ---- stage 1: x_proj^T = relu(W_in @ xT + b_in) ----
        xp = xp_pool.tile([P, PAD + T], F32R)
        nc.vector.tensor_copy(out=xp[:, 0:PAD], in_=zpad[:])
        for c in range(NCHUNK):
            ps1 = ps1_pool.tile([P, CHUNK], F32)
            nc.tensor.matmul(
                ps1[:], wi[:], xTr[:, c * CHUNK:(c + 1) * CHUNK],
                start=True, stop=True,
            )
            nc.scalar.activation(
                out=xp[:, PAD + c * CHUNK: PAD + (c + 1) * CHUNK],
                in_=ps1[:],
                func=mybir.ActivationFunctionType.Relu,
                bias=bi[:],
            )

        # ---- stage 2 (fused): outT[o, t] = sum_k F_k^T xp^T[:, t-k] ----
        for c in range(NCHUNK):
            pso = pso_pool.tile([OUT, CHUNK], F32)
            for k in range(n_lags):
                base = PAD + c * CHUNK - k
                nc.tensor.matmul(
                    pso[:], fk[:, k * OUT:(k + 1) * OUT],
                    xp[:, base: base + CHUNK],
                    start=(k == 0), stop=(k == n_lags - 1),
                )
            # ---- 32x32 block transpose + strided DMA -> [512, 64] ----
            bt = bt_pool.tile([OUT, CHUNK], F32)
            nc.vector.transpose(out=bt[:], in_=pso[:])
            for ob in range(OUT // 32):
                sb_view = bt[32 * ob:32 * (ob + 1), :].rearrange(
                    "ti (tb oi) -> ti tb oi", oi=32)
                d_view = out[b, c * CHUNK:(c + 1) * CHUNK,
                             32 * ob:32 * (ob + 1)].rearrange(
                    "(tb ti) oi -> ti tb oi", ti=32)
                nc.sync.dma_start(out=d_view, in_=sb_view)


def _n_lags(a: np.ndarray) -> int:
    amax = float(np.abs(a).max())
    if amax >= 1.0:
        return 16
    if amax <= 0.0:
        return 2
    k = int(np.ceil(np.log(2e-6) / np.log(amax)))
    return max(2, min(16, k))


def _prepare(x_seq, a, B, C, W_in, b_in, W_out, b_out):
    """Host-side folding + per-core input maps."""
    n_lags = _n_lags(a)
    a64 = a.astype(np.float64)
    B64 = B.astype(np.float64)
    C64 = C.astype(np.float64)
    CW64 = C64 @ W_out.T.astype(np.float64)                # [H, OUT]
    fks = np.stack(
        [(B64 * (a64 ** k)[None, :]) @ CW64 for k in range(n_lags)]
    ).astype(np.float32)                                   # [K, P, OUT]
    shared = {
        "wfold": np.ascontiguousarray(fks),
        "w_inT": np.ascontiguousarray(W_in.T.astype(np.float32)),
        "b_in": np.ascontiguousarray(b_in.astype(np.float32).reshape(P, 1)),
    }
    xT = np.ascontiguousarray(
        np.swapaxes(x_seq.astype(np.float32), 1, 2)
    )                                                      # [B, IN, T]
    in_maps = []
    for c in range(NCORES):
        m = dict(shared)
        m["x"] = xT[c * BL:(c + 1) * BL]
        in_maps.append(m)
    return n_lags, in_maps


def get_program(n_lags: int, reps: int = 1):
    key = (n_lags, reps)
    if key not in _programs:
        _programs[key] = _build(n_lags, reps)
    return _programs[key]


def kernel(x_seq, a, B, C, W_in, b_in, W_out, b_out):
    n_lags, in_maps = _prepare(x_seq, a, B, C, W_in, b_in, W_out, b_out)
    nc = get_program(n_lags)
    res = run_bass_kernel_spmd(nc, in_maps, list(range(NCORES)))
    out = np.concatenate([res.results[c]["out"] for c in range(NCORES)], axis=0)
    out = out.astype(np.float32)
    if np.any(b_out):
        out = out + b_out.astype(np.float32).reshape(1, 1, OUT)
    return out
